# revision 12
# baseline (speedup 1.0000x reference)
"""A3TGCN (GNN message passing) distributed Bass kernel for 8 TRN2 NeuronCores.

Math restructure (reference-equivalent):
  S = A_norm @ Xp          where Xp[n, p*32+f] = X[n, f, p]  (one sparse agg, 384 feats)
  S[d] = dinv[d] * ( sum_{e: dst=d} w_e * Y[src_e]  +  dinv[d] * Xp[d] ),  Y[n] = dinv[n]*Xp[n]
  Per period p (cols p*32..p*32+32 of S):
    Z  = sigmoid(S_p @ (Wz@Wlz_top) + (bz@Wlz_top + blz) + H @ Wlz_bot)
    R  = sigmoid(S_p @ (Wr@Wlr_top) + (br@Wlr_top + blr) + H @ Wlr_bot)
    Ht = tanh  (S_p @ (Wh@Wlh_top) + (bh@Wlh_top + blh) + (H*R) @ Wlh_bot)
    Hp = Z*H + (1-Z)*Ht ;  out = sum_p softmax(att)[p] * Hp

Sharding: core c owns dst nodes [c*NSHARD, (c+1)*NSHARD). Host groups edges by
(dst-core, dst-window of 128, src-chunk), dst-window sorted. Device:
  pass A: deg via one-hot matmul segment-reduce -> dinv (local)
  Y build (local shard, bf16) -> AllGather Y
  pass B: dma_gather Y rows per edge; lhsT = onehot(dstrel)*w, rhs = Y rows;
          PSUM-accumulate per 128-dst window; epilogue adds self loop + dinv
          scaling; PE-transpose into feature-major S_T
  phase 2: dense gates via block-diag matmuls, attention-weighted PE reduce.
All float math happens on device; host does only integer index prep / layout.
"""

import os
import sys
import numpy as np

for _p in ("/opt/trn_rl_repo",):
    if _p not in sys.path and os.path.isdir(_p):
        sys.path.insert(0, _p)

import concourse.bass as bass
import concourse.bacc as bacc
import concourse.mybir as mybir
import concourse.tile as tile
from concourse.bass_utils import run_bass_kernel_spmd
from concourse.masks import make_identity

F32 = mybir.dt.float32
BF16 = mybir.dt.bfloat16
I16 = mybir.dt.int16
I32 = mybir.dt.int32
AX = mybir.AxisListType
ALU = mybir.AluOpType
ACTF = mybir.ActivationFunctionType

NC_CORES = 8
NCHUNK = 4  # source-node chunks (gather tables); table rows = 2*NPAD <= 32767

last_exec_time_ns = None
last_results = None


def _round_up(x, m):
    return (x + m - 1) // m * m


class Prep:
    """Host-side integer/layout preprocessing (index math only)."""

    def __init__(self, n_nodes, f_in, periods, edge_src, edge_dst, edge_w):
        assert n_nodes % NC_CORES == 0
        self.N = n_nodes
        self.F = f_in
        self.P = periods
        self.D = f_in * periods
        self.NSHARD = n_nodes // NC_CORES
        self.NPAD = _round_up(self.NSHARD, 128)
        self.W = self.NPAD // 128
        self.CPC = NC_CORES // NCHUNK
        self.TROWS = self.CPC * self.NPAD
        assert self.TROWS <= 32767

        src = np.asarray(edge_src, np.int64)
        dst = np.asarray(edge_dst, np.int64)
        w = np.asarray(edge_w, np.float32)

        core = dst // self.NSHARD
        loc = dst % self.NSHARD
        win = loc // 128
        drel = (loc % 128).astype(np.float32)
        core_s = src // self.NSHARD
        tidx = ((core_s % self.CPC) * self.NPAD + (src % self.NSHARD)).astype(
            np.int16
        )
        chunk = core_s // self.CPC

        key = ((core * self.W) + win) * NCHUNK + chunk
        order = np.argsort(key, kind="stable")
        tidx_s = tidx[order]
        drel_s = drel[order]
        w_s = w[order]

        cnt = np.bincount(key, minlength=NC_CORES * self.W * NCHUNK).reshape(
            NC_CORES, self.W, NCHUNK
        )
        starts = np.zeros(NC_CORES * self.W * NCHUNK + 1, dtype=np.int64)
        np.cumsum(cnt.reshape(-1), out=starts[1:])

        V = cnt.max(axis=0)  # [W, NCHUNK] valid idx count (same across cores)
        C = np.where(V > 0, ((V + 127) // 128) * 128, 0)

        self.V = V
        self.C = C
        self.T = C // 128
        self.TW = self.T.sum(axis=1)

        wd_len = int(C.sum())
        idx_len = 128 * int((C // 16).sum())
        self.wd_len = max(wd_len, 128)
        self.idx_len = max(idx_len, 128)

        self.wstream = np.zeros((NC_CORES, self.wd_len), dtype=np.float32)
        self.dstream = np.zeros((NC_CORES, self.wd_len), dtype=np.float32)
        self.istream = np.full((NC_CORES, self.idx_len), -1, dtype=np.int16)

        self.wd_off = np.zeros((self.W, NCHUNK), dtype=np.int64)
        self.idx_off = np.zeros((self.W, NCHUNK), dtype=np.int64)
        self.win_wd_off = np.zeros(self.W, dtype=np.int64)
        o1 = 0
        o2 = 0
        for wi in range(self.W):
            self.win_wd_off[wi] = o1
            for g in range(NCHUNK):
                self.wd_off[wi, g] = o1
                self.idx_off[wi, g] = o2
                o1 += int(C[wi, g])
                o2 += 128 * (int(C[wi, g]) // 16)

        for c in range(NC_CORES):
            for wi in range(self.W):
                wblocks = []
                dblocks = []
                for g in range(NCHUNK):
                    Cg = int(C[wi, g])
                    if Cg == 0:
                        continue
                    k = (c * self.W + wi) * NCHUNK + g
                    s0, s1 = int(starts[k]), int(starts[k + 1])
                    ncg = s1 - s0
                    ivals = np.zeros(Cg, dtype=np.int16)
                    wvals = np.zeros(Cg, dtype=np.float32)
                    dvals = np.zeros(Cg, dtype=np.float32)
                    ivals[:ncg] = tidx_s[s0:s1]
                    wvals[:ncg] = w_s[s0:s1]
                    dvals[:ncg] = drel_s[s0:s1]
                    Tg = Cg // 128
                    wblocks.append(wvals.reshape(Tg, 128).T)
                    dblocks.append(dvals.reshape(Tg, 128).T)
                    oi = int(self.idx_off[wi, g])
                    blk = np.tile(ivals.reshape(Cg // 16, 16).T, (8, 1))
                    self.istream[c, oi : oi + 128 * (Cg // 16)] = blk.reshape(-1)
                if wblocks:
                    warr = np.hstack(wblocks)  # [128, TW_w], col order = group order
                    darr = np.hstack(dblocks)
                    o = int(self.win_wd_off[wi])
                    nsl = 128 * warr.shape[1]
                    self.wstream[c, o : o + nsl] = warr.reshape(-1)
                    self.dstream[c, o : o + nsl] = darr.reshape(-1)


def build_kernel(prep: "Prep", h_nonzero: bool):
    D, P, F = prep.D, prep.P, prep.F
    NPAD, W = prep.NPAD, prep.W
    assert P % 4 == 0 and D % 128 == 0
    NG = P // 4  # period groups of 4 (128 partitions each)
    DJ = D // 128
    assert DJ == NG
    TROWS = prep.TROWS

    nc = bacc.Bacc("TRN2", target_bir_lowering=False)

    xsh = nc.declare_dram_parameter("xsh", [NPAD, D], F32, isOutput=False)
    hsh = nc.declare_dram_parameter("hsh", [NPAD, F], F32, isOutput=False)
    wstream = nc.declare_dram_parameter("wstream", [prep.wd_len], F32, isOutput=False)
    dstream = nc.declare_dram_parameter("dstream", [prep.wd_len], F32, isOutput=False)
    istream = nc.declare_dram_parameter("istream", [prep.idx_len], I16, isOutput=False)
    att = nc.declare_dram_parameter("att", [1, P], F32, isOutput=False)
    wts = {}
    for nm in ("wz", "wr", "wh", "wlzt", "wlrt", "wlht", "wlzb", "wlrb", "wlhb"):
        wts[nm] = nc.declare_dram_parameter(nm, [F, F], F32, isOutput=False)
    bias_p = {}
    for nm in ("bz", "br", "bh", "blz", "blr", "blh"):
        bias_p[nm] = nc.declare_dram_parameter(nm, [1, F], F32, isOutput=False)
    sel12 = nc.declare_dram_parameter("sel12", [NG * P, 128], F32, isOutput=False)
    i4x32 = nc.declare_dram_parameter("i4x32", [128, F], F32, isOutput=False)
    outp = nc.declare_dram_parameter("outp", [NPAD, F], F32, isOutput=True)
    DEBUG = bool(os.environ.get("K_DEBUG"))
    if DEBUG:
        dbg_s = nc.declare_dram_parameter("dbg_s", [NPAD, D], F32, isOutput=True)
        dbg_dinv = nc.declare_dram_parameter("dbg_dinv", [128, W], F32, isOutput=True)

    ysh = nc.dram_tensor("ysh", [NPAD, D], BF16)
    yfull = nc.dram_tensor("yfull", [NC_CORES * NPAD, D], BF16, addr_space="Shared")

    with tile.TileContext(nc) as tc:
        with (
            tc.tile_pool(name="const", bufs=1) as constp,
            tc.tile_pool(name="longl", bufs=1) as longp,
        ):
            ident = constp.tile([128, 128], F32, tag="ident")
            make_identity(nc, ident[:])
            iota_i = constp.tile([128, 128], I32, tag="iota_i")
            nc.gpsimd.iota(iota_i[:], pattern=[[1, 128]], channel_multiplier=0)
            iota_f = constp.tile([128, 128], F32, tag="iota_f")
            nc.vector.tensor_copy(iota_f[:], iota_i[:])

            dinv_sb = longp.tile([128, W], F32, tag="dinv")
            st_t = [longp.tile([128, NPAD], BF16, tag=f"st{j}", name=f"st{j}") for j in range(NG)]

            # ---------------- PASS A: degree -> dinv ----------------
            with (
                tc.tile_pool(name="pa_wd", bufs=3) as pwd,
                tc.tile_pool(name="pa_sel", bufs=4) as psel,
                tc.tile_pool(name="pa_ps", bufs=2, space="PSUM") as pps,
                tc.tile_pool(name="pa_tmp", bufs=2) as ptmp,
            ):
                for wi in range(W):
                    TWw = int(prep.TW[wi])
                    if TWw == 0:
                        nc.vector.memset(dinv_sb[:, wi : wi + 1], 1.0)
                        continue
                    o = int(prep.win_wd_off[wi])
                    wcol = pwd.tile([128, TWw], F32, tag="wcol")
                    nc.sync.dma_start(
                        out=wcol[:],
                        in_=wstream[o : o + 128 * TWw].rearrange("(p t) -> p t", p=128),
                    )
                    dcol = pwd.tile([128, TWw], F32, tag="dcol")
                    nc.sync.dma_start(
                        out=dcol[:],
                        in_=dstream[o : o + 128 * TWw].rearrange("(p t) -> p t", p=128),
                    )
                    wbf = pwd.tile([128, TWw], BF16, tag="wbf")
                    nc.vector.tensor_copy(wbf[:], wcol[:])
                    ps = pps.tile([128, 1], F32, tag="psA")
                    for t in range(TWw):
                        sel = psel.tile([128, 128], BF16, tag="sel")
                        nc.vector.tensor_scalar(
                            sel[:], iota_f[:], dcol[:, t : t + 1], None, ALU.is_equal
                        )
                        nc.tensor.matmul(
                            ps[:],
                            lhsT=sel[:],
                            rhs=wbf[:, t : t + 1],
                            start=(t == 0),
                            stop=(t == TWw - 1),
                        )
                    sq = ptmp.tile([128, 1], F32, tag="sq")
                    nc.scalar.activation(sq[:], ps[:], ACTF.Sqrt, bias=1.0)
                    nc.vector.reciprocal(dinv_sb[:, wi : wi + 1], sq[:])

            # ---------------- Y build + AllGather ----------------
            with tc.tile_pool(name="yb", bufs=3) as pyb:
                for wi in range(W):
                    xw = pyb.tile([128, D], F32, tag="xw")
                    nc.sync.dma_start(out=xw[:], in_=xsh[wi * 128 : (wi + 1) * 128, :])
                    yb = pyb.tile([128, D], BF16, tag="yb")
                    nc.vector.tensor_scalar(
                        yb[:], xw[:], dinv_sb[:, wi : wi + 1], None, ALU.mult
                    )
                    nc.sync.dma_start(out=ysh[wi * 128 : (wi + 1) * 128, :], in_=yb[:])
            nc.gpsimd.collective_compute(
                "AllGather",
                ALU.bypass,
                replica_groups=[list(range(NC_CORES))],
                ins=[ysh[:, :].opt()],
                outs=[yfull[:, :].opt()],
            )

            # ---------------- PASS B: edge aggregation ----------------
            with (
                tc.tile_pool(name="pb_wd", bufs=3) as pwd,
                tc.tile_pool(name="pb_idx", bufs=3) as pidx,
                tc.tile_pool(name="pb_y", bufs=3) as pyg,
                tc.tile_pool(name="pb_sel", bufs=4) as psel,
                tc.tile_pool(name="pb_ps", bufs=2, space="PSUM") as pps,
                tc.tile_pool(name="pb_tp", bufs=2, space="PSUM") as ptp,
                tc.tile_pool(name="pb_tmp", bufs=2) as ptmp,
            ):
                for wi in range(W):
                    TWw = int(prep.TW[wi])
                    xw = ptmp.tile([128, D], F32, tag="xw")
                    nc.sync.dma_start(out=xw[:], in_=xsh[wi * 128 : (wi + 1) * 128, :])
                    s_f = ptmp.tile([128, D], F32, tag="s_f")
                    nc.vector.tensor_scalar(
                        s_f[:], xw[:], dinv_sb[:, wi : wi + 1], None, ALU.mult
                    )
                    if TWw > 0:
                        o = int(prep.win_wd_off[wi])
                        wcol = pwd.tile([128, TWw], F32, tag="wcol")
                        nc.sync.dma_start(
                            out=wcol[:],
                            in_=wstream[o : o + 128 * TWw].rearrange(
                                "(p t) -> p t", p=128
                            ),
                        )
                        dcol = pwd.tile([128, TWw], F32, tag="dcol")
                        nc.sync.dma_start(
                            out=dcol[:],
                            in_=dstream[o : o + 128 * TWw].rearrange(
                                "(p t) -> p t", p=128
                            ),
                        )
                        ps = pps.tile([128, D], F32, tag="psB")
                        kk = 0
                        tcol = 0
                        for g in range(NCHUNK):
                            Tg = int(prep.T[wi, g])
                            if Tg == 0:
                                continue
                            Cg = int(prep.C[wi, g])
                            oi = int(prep.idx_off[wi, g])
                            idxt = pidx.tile([128, Cg // 16], I16, tag="idxt")
                            nc.sync.dma_start(
                                out=idxt[:],
                                in_=istream[oi : oi + 128 * (Cg // 16)].rearrange(
                                    "(p s) -> p s", p=128
                                ),
                            )
                            yg = pyg.tile([128, Tg, D], BF16, tag="yg")
                            nc.gpsimd.dma_gather(
                                out_ap=yg[:],
                                in_ap=yfull[g * TROWS : (g + 1) * TROWS, :],
                                idxs_ap=idxt[:],
                                num_idxs=Cg,
                                num_idxs_reg=Cg,
                                elem_size=D,
                            )
                            for t in range(Tg):
                                sel = psel.tile([128, 128], BF16, tag="sel")
                                nc.vector.tensor_scalar(
                                    sel[:],
                                    iota_f[:],
                                    dcol[:, tcol : tcol + 1],
                                    wcol[:, tcol : tcol + 1],
                                    ALU.is_equal,
                                    ALU.mult,
                                )
                                nc.tensor.matmul(
                                    ps[:],
                                    lhsT=sel[:],
                                    rhs=yg[:, t, :],
                                    start=(kk == 0),
                                    stop=(kk == TWw - 1),
                                )
                                kk += 1
                                tcol += 1
                        nc.vector.tensor_tensor(
                            out=s_f[:], in0=ps[:], in1=s_f[:], op=ALU.add
                        )
                    nc.vector.tensor_scalar(
                        s_f[:], s_f[:], dinv_sb[:, wi : wi + 1], None, ALU.mult
                    )
                    if DEBUG:
                        nc.sync.dma_start(
                            out=dbg_s[wi * 128 : (wi + 1) * 128, :], in_=s_f[:]
                        )
                    for j in range(DJ):
                        tp = ptp.tile([128, 128], F32, tag="tpB")
                        nc.tensor.transpose(
                            tp[:], s_f[:, j * 128 : (j + 1) * 128], ident[:]
                        )
                        nc.vector.tensor_copy(
                            st_t[j][:, wi * 128 : (wi + 1) * 128], tp[:]
                        )

            # ---------------- PHASE 2: gates ----------------
            with (
                tc.tile_pool(name="p2c", bufs=1) as p2c,
                tc.tile_pool(name="p2ps", bufs=2, space="PSUM") as p2ps,
                tc.tile_pool(name="p2acc", bufs=2, space="PSUM") as p2acc,
                tc.tile_pool(name="p2tp", bufs=2, space="PSUM") as p2tp,
                tc.tile_pool(name="p2t", bufs=3) as p2t,
            ):
                def ps5():
                    return p2ps.tile([128, 512], F32, tag="ps5", name="ps5")

                def tp128():
                    return p2tp.tile([128, 128], F32, tag="tp", name="tp")

                def to_bf(ap, shape, tag):
                    t = p2c.tile(shape, BF16, tag=tag, name=tag)
                    nc.vector.tensor_copy(t[:], ap)
                    return t

                blk = {}
                w4b = {}
                wlt_bfs = {}
                for gz in ("z", "r", "h"):
                    wsb = p2c.tile([F, F], F32, tag="w_" + gz)
                    nc.sync.dma_start(out=wsb[:], in_=wts["w" + gz][:, :])
                    wltsb = p2c.tile([F, F], F32, tag="wlt_" + gz)
                    nc.sync.dma_start(out=wltsb[:], in_=wts["wl" + gz + "t"][:, :])
                    wlbsb = p2c.tile([F, F], F32, tag="wlb_" + gz)
                    nc.sync.dma_start(out=wlbsb[:], in_=wts["wl" + gz + "b"][:, :])
                    pst = tp128()
                    nc.tensor.transpose(pst[:F, :F], wsb[:], ident[:F, :F])
                    wT_bf = to_bf(pst[:F, :F], [F, F], "wT" + gz)
                    wlt_bf = to_bf(wltsb[:], [F, F], "wltb" + gz)
                    wlt_bfs[gz] = wlt_bf
                    psa = ps5()
                    nc.tensor.matmul(
                        psa[:F, :F], lhsT=wT_bf[:], rhs=wlt_bf[:], start=True, stop=True
                    )
                    a_bf = to_bf(psa[:F, :F], [F, F], "a" + gz)
                    blkA = p2c.tile([128, 128], BF16, tag="blkA" + gz)
                    nc.vector.memset(blkA[:], 0.0)
                    for pp in range(4):
                        nc.vector.tensor_copy(
                            blkA[pp * F : (pp + 1) * F, pp * F : (pp + 1) * F], a_bf[:]
                        )
                    blk[gz] = blkA
                    wlb_bf = to_bf(wlbsb[:], [F, F], "wlbb" + gz)
                    if gz == "h":
                        blkB = p2c.tile([128, 128], BF16, tag="blkB" + gz)
                        nc.vector.memset(blkB[:], 0.0)
                        for pp in range(4):
                            nc.vector.tensor_copy(
                                blkB[pp * F : (pp + 1) * F, pp * F : (pp + 1) * F],
                                wlb_bf[:],
                            )
                        w4b[gz] = blkB
                    else:
                        w4 = p2c.tile([F, 128], BF16, tag="w4" + gz)
                        for pp in range(4):
                            nc.vector.tensor_copy(w4[:, pp * F : (pp + 1) * F], wlb_bf[:])
                        w4b[gz] = w4

                one1b = p2c.tile([1, 1], BF16, tag="one1b")
                nc.vector.memset(one1b[:], 1.0)
                bias4 = {}
                for gz in ("z", "r", "h"):
                    bsb = p2c.tile([1, F], F32, tag="bsb" + gz)
                    nc.sync.dma_start(out=bsb[:], in_=bias_p["b" + gz][:, :])
                    blsb = p2c.tile([1, F], F32, tag="blsb" + gz)
                    nc.sync.dma_start(out=blsb[:], in_=bias_p["bl" + gz][:, :])
                    b_bf = to_bf(bsb[:], [1, F], "bbf" + gz)
                    psb = tp128()
                    nc.tensor.matmul(
                        psb[:F, :1], lhsT=b_bf[:], rhs=one1b[:], start=True, stop=True
                    )
                    bT_bf = to_bf(psb[:F, :1], [F, 1], "bT" + gz)
                    psr = ps5()
                    nc.tensor.matmul(
                        psr[:1, :F], lhsT=bT_bf[:], rhs=wlt_bfs[gz][:], start=True,
                        stop=True,
                    )
                    crow = p2c.tile([1, F], F32, tag="crow" + gz)
                    nc.vector.tensor_tensor(
                        out=crow[:], in0=psr[:1, :F], in1=blsb[:], op=ALU.add
                    )
                    cbf = to_bf(crow[:], [1, F], "cbf" + gz)
                    psc = tp128()
                    nc.tensor.matmul(
                        psc[:F, :1], lhsT=cbf[:], rhs=one1b[:], start=True, stop=True
                    )
                    b4 = p2c.tile([128, 1], F32, tag="b4" + gz)
                    for pp in range(4):
                        nc.vector.tensor_copy(b4[pp * F : (pp + 1) * F, :], psc[:F, :1])
                    bias4[gz] = b4

                attsb = p2c.tile([1, P], F32, tag="attsb")
                nc.sync.dma_start(out=attsb[:], in_=att[:, :])
                ex = p2c.tile([1, P], F32, tag="ex")
                nc.scalar.activation(ex[:], attsb[:], ACTF.Exp)
                sm = p2c.tile([1, 1], F32, tag="sm")
                nc.vector.tensor_reduce(sm[:], ex[:], axis=AX.X, op=ALU.add)
                rsm = p2c.tile([1, 1], F32, tag="rsm")
                nc.vector.reciprocal(rsm[:], sm[:])
                probs = p2c.tile([1, P], BF16, tag="probs")
                nc.vector.tensor_scalar(probs[:], ex[:], rsm[:], None, ALU.mult)
                pst2 = tp128()
                nc.tensor.matmul(
                    pst2[:P, :1], lhsT=probs[:], rhs=one1b[:], start=True, stop=True
                )
                probsT = to_bf(pst2[:P, :1], [P, 1], "probsT")
                i4sb = p2c.tile([128, F], F32, tag="i4sb")
                nc.sync.dma_start(out=i4sb[:], in_=i4x32[:, :])
                redg = []
                for G in range(NG):
                    selg = p2c.tile([P, 128], F32, tag="selg")
                    nc.sync.dma_start(out=selg[:], in_=sel12[G * P : (G + 1) * P, :])
                    selg_bf = to_bf(selg[:], [P, 128], "selgb")
                    pspc = tp128()
                    nc.tensor.matmul(
                        pspc[:, :1], lhsT=selg_bf[:], rhs=probsT[:], start=True,
                        stop=True,
                    )
                    pcol = p2c.tile([128, 1], F32, tag=f"pcol{G}")
                    nc.vector.tensor_copy(pcol[:], pspc[:, :1])
                    rg = p2c.tile([128, F], BF16, tag=f"redg{G}")
                    nc.vector.tensor_scalar(rg[:], i4sb[:], pcol[:], None, ALU.mult)
                    redg.append(rg)

                if h_nonzero:
                    h_t = longp.tile([F, NPAD], BF16, tag="h_t")
                    h4 = longp.tile([128, NPAD], BF16, tag="h4")
                    for wi in range(W):
                        hw = p2t.tile([128, F], F32, tag="hw")
                        nc.sync.dma_start(
                            out=hw[:], in_=hsh[wi * 128 : (wi + 1) * 128, :]
                        )
                        psh = tp128()
                        nc.tensor.transpose(psh[:F, :], hw[:], ident[:])
                        nc.vector.tensor_copy(
                            h_t[:, wi * 128 : (wi + 1) * 128], psh[:F, :]
                        )
                    for pp in range(4):
                        nc.vector.tensor_copy(h4[pp * F : (pp + 1) * F, :], h_t[:])

                NCHW = 512
                nchunks = (NPAD + NCHW - 1) // NCHW
                outstage = longp.tile([128, W * F], F32, tag="outstage")
                for ci in range(nchunks):
                    c0 = ci * NCHW
                    L = min(NCHW, NPAD - c0)
                    acc = p2acc.tile([F, NCHW], F32, tag="acc")
                    for G in range(NG):
                        rhs_s = st_t[G][:, c0 : c0 + L]
                        psz = ps5()
                        nc.tensor.matmul(
                            psz[:, :L], lhsT=blk["z"][:], rhs=rhs_s,
                            start=True, stop=not h_nonzero,
                        )
                        if h_nonzero:
                            nc.tensor.matmul(
                                psz[:, :L], lhsT=w4b["z"][:], rhs=h_t[:, c0 : c0 + L],
                                start=False, stop=True,
                            )
                        zt = p2t.tile([128, NCHW], BF16, tag="zt")
                        nc.scalar.activation(
                            zt[:, :L], psz[:, :L], ACTF.Sigmoid, bias=bias4["z"][:]
                        )
                        psh2 = ps5()
                        nc.tensor.matmul(
                            psh2[:, :L], lhsT=blk["h"][:], rhs=rhs_s,
                            start=True, stop=not h_nonzero,
                        )
                        if h_nonzero:
                            psr2 = ps5()
                            nc.tensor.matmul(
                                psr2[:, :L], lhsT=blk["r"][:], rhs=rhs_s,
                                start=True, stop=False,
                            )
                            nc.tensor.matmul(
                                psr2[:, :L], lhsT=w4b["r"][:], rhs=h_t[:, c0 : c0 + L],
                                start=False, stop=True,
                            )
                            rt = p2t.tile([128, NCHW], BF16, tag="rt")
                            nc.scalar.activation(
                                rt[:, :L], psr2[:, :L], ACTF.Sigmoid, bias=bias4["r"][:]
                            )
                            hr = p2t.tile([128, NCHW], BF16, tag="hr")
                            nc.vector.tensor_tensor(
                                out=hr[:, :L], in0=rt[:, :L], in1=h4[:, c0 : c0 + L],
                                op=ALU.mult,
                            )
                            nc.tensor.matmul(
                                psh2[:, :L], lhsT=w4b["h"][:], rhs=hr[:, :L],
                                start=False, stop=True,
                            )
                        ht = p2t.tile([128, NCHW], BF16, tag="ht")
                        nc.scalar.activation(
                            ht[:, :L], psh2[:, :L], ACTF.Tanh, bias=bias4["h"][:]
                        )
                        hp = p2t.tile([128, NCHW], BF16, tag="hp")
                        d1 = p2t.tile([128, NCHW], BF16, tag="d1")
                        if h_nonzero:
                            nc.vector.tensor_tensor(
                                out=d1[:, :L], in0=h4[:, c0 : c0 + L], in1=ht[:, :L],
                                op=ALU.subtract,
                            )
                            nc.vector.tensor_tensor(
                                out=d1[:, :L], in0=zt[:, :L], in1=d1[:, :L], op=ALU.mult
                            )
                            nc.vector.tensor_tensor(
                                out=hp[:, :L], in0=ht[:, :L], in1=d1[:, :L], op=ALU.add
                            )
                        else:
                            nc.vector.tensor_tensor(
                                out=d1[:, :L], in0=zt[:, :L], in1=ht[:, :L], op=ALU.mult
                            )
                            nc.vector.tensor_tensor(
                                out=hp[:, :L], in0=ht[:, :L], in1=d1[:, :L],
                                op=ALU.subtract,
                            )
                        nc.tensor.matmul(
                            acc[:, :L], lhsT=redg[G][:], rhs=hp[:, :L],
                            start=(G == 0), stop=(G == NG - 1),
                        )
                    hacc = p2t.tile([F, NCHW], F32, tag="hacc")
                    nc.vector.tensor_copy(hacc[:, :L], acc[:, :L])
                    for j in range(L // 128):
                        tpp = tp128()
                        nc.tensor.transpose(
                            tpp[:, :F], hacc[:, j * 128 : (j + 1) * 128],
                            ident[:F, :F],
                        )
                        bidx = (c0 // 128) + j
                        nc.vector.tensor_copy(
                            outstage[:, bidx * F : (bidx + 1) * F], tpp[:, :F]
                        )
                nc.sync.dma_start(
                    out=outp[:, :].rearrange("(b p) f -> p b f", p=128),
                    in_=outstage[:].rearrange("p (b f) -> p b f", f=F),
                )
                if DEBUG:
                    nc.sync.dma_start(out=dbg_dinv[:, :], in_=dinv_sb[:])
    nc.compile()
    return nc


def _host_prep_inputs(X, edge_index, edge_weight, H, attention, wdict):
    X = np.asarray(X, np.float32)
    N, F_IN, P = X.shape
    prep = Prep(
        N, F_IN, P, np.asarray(edge_index)[0], np.asarray(edge_index)[1],
        np.asarray(edge_weight),
    )
    NPAD, NSHARD, D = prep.NPAD, prep.NSHARD, prep.D
    Xp = np.ascontiguousarray(np.transpose(X, (0, 2, 1))).reshape(N, D)
    Hf = np.asarray(H, np.float32)
    h_nonzero = bool(np.any(Hf))
    NG = P // 4
    sel12 = np.zeros((NG * P, 128), np.float32)
    for G in range(NG):
        for q in range(128):
            sel12[G * P + G * 4 + q // F_IN, q] = 1.0
    i4x32 = np.zeros((128, F_IN), np.float32)
    for q in range(128):
        i4x32[q, q % F_IN] = 1.0

    in_maps = []
    for c in range(NC_CORES):
        xs = np.zeros((NPAD, D), np.float32)
        xs[:NSHARD] = Xp[c * NSHARD : (c + 1) * NSHARD]
        hs = np.zeros((NPAD, F_IN), np.float32)
        hs[:NSHARD] = Hf[c * NSHARD : (c + 1) * NSHARD]
        m = dict(
            xsh=xs, hsh=hs,
            wstream=prep.wstream[c], dstream=prep.dstream[c],
            istream=prep.istream[c],
            att=np.asarray(attention, np.float32).reshape(1, P),
            sel12=sel12, i4x32=i4x32,
            wz=np.asarray(wdict["Wz"], np.float32),
            wr=np.asarray(wdict["Wr"], np.float32),
            wh=np.asarray(wdict["Wh"], np.float32),
            wlzt=np.ascontiguousarray(np.asarray(wdict["Wlz"], np.float32)[:F_IN]),
            wlrt=np.ascontiguousarray(np.asarray(wdict["Wlr"], np.float32)[:F_IN]),
            wlht=np.ascontiguousarray(np.asarray(wdict["Wlh"], np.float32)[:F_IN]),
            wlzb=np.ascontiguousarray(np.asarray(wdict["Wlz"], np.float32)[F_IN:]),
            wlrb=np.ascontiguousarray(np.asarray(wdict["Wlr"], np.float32)[F_IN:]),
            wlhb=np.ascontiguousarray(np.asarray(wdict["Wlh"], np.float32)[F_IN:]),
            bz=np.asarray(wdict["bz"], np.float32).reshape(1, F_IN),
            br=np.asarray(wdict["br"], np.float32).reshape(1, F_IN),
            bh=np.asarray(wdict["bh"], np.float32).reshape(1, F_IN),
            blz=np.asarray(wdict["blz"], np.float32).reshape(1, F_IN),
            blr=np.asarray(wdict["blr"], np.float32).reshape(1, F_IN),
            blh=np.asarray(wdict["blh"], np.float32).reshape(1, F_IN),
        )
        in_maps.append(m)
    return prep, in_maps, h_nonzero


def kernel(X, edge_index, edge_weight, H, attention, Wz, bz, Wr, br, Wh, bh,
           Wlz, blz, Wlr, blr, Wlh, blh):
    global last_exec_time_ns, last_results
    wdict = dict(Wz=Wz, bz=bz, Wr=Wr, br=br, Wh=Wh, bh=bh, Wlz=Wlz, blz=blz,
                 Wlr=Wlr, blr=blr, Wlh=Wlh, blh=blh)
    prep, in_maps, h_nonzero = _host_prep_inputs(
        X, edge_index, edge_weight, H, attention, wdict
    )
    nc = build_kernel(prep, h_nonzero)
    trace = bool(os.environ.get("BASS_TRACE"))
    res = run_bass_kernel_spmd(
        nc, in_maps, core_ids=list(range(NC_CORES)), trace=trace
    )
    last_exec_time_ns = res.exec_time_ns
    last_results = res
    NSHARD = prep.NSHARD
    out = np.concatenate(
        [np.asarray(res.results[c]["outp"])[:NSHARD] for c in range(NC_CORES)],
        axis=0,
    )
    return out.astype(np.float32)
